# revision 41
# baseline (speedup 1.0000x reference)
"""Trainium2 Bass kernel: windowed attention with dynamic positional bias.

out = softmax(scale*q@k^T + bias) @ v per (batch, head); the tiny pos-bias
MLP table (43x6) is computed on host and folded into the QK matmul via
augmented contraction channels:

  qaug[n] . kaug[m] = scale*q[n].k[m] + pos[s(n)-s(m)+OFF]     (exact)

qaug = [qs_hi, qs_hi, qs_lo, pos rows] / kaug = [k_hi, k_lo, k_hi,
onehot(s(m))] in bf16, K=118.  Matmuls with K<~118 partition rows are NOT
faster and can be catastrophically slower on this stack: K=54
row-alternating tiles measured 2.6x slower, K=65 full-base 1.6x slower,
despite identical streaming width — keep operands near 128 partitions.
The hi/lo split is numerically free (rel_err 1.31e-2 vs 1.50e-2 plain).

Softmax exp is split between ScalarE (exact exp LUT over S' chunks 0-2
in spA) and VectorE (Schraudolph fast-exp over chunk 3 in spB: I16 =
l*184.665 + 16250.5 bit-reinterpreted as bf16 = e^l, ~2.5% sawtooth on
25% of the weights).  The split must stay at whole key-chunks: a
column-split of a chunk divides by QUERY, concentrating fast-exp to 50%
of some query rows' softmax terms (measured rel_err 2.31e-2, FAIL), and
sharing an spA bank between ACT and DVE readers also serializes the two
engines (Tile's bank tracker) — both hit when A_COLS=1280 was tried.

Layout (vs the 115us baseline):
- DMA is batched into ~1.5MB mega-transfers: each transfer pays ~2us of
  fixed completion latency on its FIFO ring, so the baseline's 12-group
  streaming (24+ transfers/ring) was latency-bound at ~70-85us/ring.
  Whole qaug/kaug/vaug stay resident in SBUF.  Rings: sync=qaug x4,
  scalar(ACT HWDGE)=kaug x4 + vaug x2, gpsimd=out stores x8.  (Moving
  loads to gpsimd/SWDGE or outs to sync measured far slower - keep this
  assignment.)
- AV runs as a 2-pair block: pair A accumulates at PSUM partitions 0-32,
  pair B at 64-96 of the SAME bank (pair B's consumed spB bank).
  start=True pending-zero clears are per-partition, so the interleaved
  chains don't clobber each other.  One DVE eviction copy [0:97] per
  block (f32 PSUM copies run 1x, so halving the copy count saves
  ~330ns/pair).
- Pair B's AV chain runs c3..c0 so its first matmul reads the DVE
  fast-exp output (ready early) instead of waiting on the ACT exp.

Data parallel: 8 cores x 8 batches = 48 (b,h) pairs/core.
"""

import sys

for _p in ("/opt/trn_rl_repo",):
    if _p not in sys.path:
        sys.path.insert(0, _p)

from contextlib import ExitStack

import ml_dtypes
import numpy as np

import concourse.bacc as bacc
import concourse.bass as bass
import concourse.tile as tile
from concourse import mybir
from concourse.bass_utils import run_bass_kernel_spmd

B, HEADS, HEAD_DIM = 64, 6, 32
NCORES = 8
BPC = B // NCORES              # batches per core
PAIRS = BPC * HEADS            # 48 (b,h) pairs per core
NBLK = PAIRS // 2              # 24 two-pair AV blocks
N = 512                        # sequence positions (h*w*d)
NAUG = 22                      # bias channels (s in 0..21)
QROWS = 128                    # qaug/kaug partition rows (hi/lo + bias = 118)
VA = HEAD_DIM + 1              # [v, ones] columns
A_COLS = 1536                  # exp cols on ACT (spA); 512 on DVE fast-exp (spB)
# q/k DMA chunks by pair range: small head so the first QK can start
# ~3.2us after the loads issue instead of ~5.6us.
QCHUNKS = [(0, 3), (4, 11), (12, 23), (24, 35), (36, 47)]
VCH = 2                        # v DMA chunks (24 pairs each)
# out stores by block range: smaller final store shrinks the post-compute
# tail (all on the gpsimd ring - sync measured slower for stores).
OSTORES = [(0, 5), (6, 11), (12, 17), (18, 21), (22, 23)]
FE_SCALE = 184.6649652337873   # 128 * log2(e)
FE_BIAS = 16250.5              # 127*128 + c; rint cast on DVE, c tuned on data

_BF16 = mybir.dt.bfloat16
_F32 = mybir.dt.float32
_I16 = mybir.dt.int16

_Exp = mybir.ActivationFunctionType.Exp
_mult = mybir.AluOpType.mult
_add = mybir.AluOpType.add


def _ln(x, g, b, eps=1e-5):
    mu = x.mean(axis=-1, keepdims=True)
    var = x.var(axis=-1, keepdims=True)
    return (x - mu) / np.sqrt(var + eps) * g + b


def _pos_table(h, w, d, pos_proj_w, pos_proj_b, ln1_g, ln1_b, w1, b1,
               ln2_g, ln2_b, w2, b2, ln3_g, ln3_b, w3, b3):
    bh = np.arange(1 - h, h, dtype=np.float32)
    bw = np.arange(1 - w, w, dtype=np.float32)
    bd = np.arange(1 - d, d, dtype=np.float32)
    mesh = np.stack(np.meshgrid(bh, bw, bd, indexing='ij')).reshape(3, -1).T
    x = mesh.astype(np.float32) @ pos_proj_w + pos_proj_b
    x = np.maximum(_ln(x, ln1_g, ln1_b), 0) @ w1 + b1
    x = np.maximum(_ln(x, ln2_g, ln2_b), 0) @ w2 + b2
    return (np.maximum(_ln(x, ln3_g, ln3_b), 0) @ w3 + b3).astype(np.float32)


def _build_device_program(loop_reps=None):
    """loop_reps: wrap the body in a device-side For_i (timing harness)."""
    nc = bacc.Bacc("TRN2", target_bir_lowering=False, debug=False)

    qf = PAIRS * N                 # qaug/kaug dram: [QROWS, qf] bf16
    vf = PAIRS * 4 * VA            # v dram: [128, vf] bf16
    of = NBLK * N                  # out dram: [66, of] f32 (2 pairs/block)

    qaug = nc.dram_tensor("qaug", [QROWS, qf], _BF16, kind="ExternalInput").ap()
    kaug = nc.dram_tensor("kaug", [QROWS, qf], _BF16, kind="ExternalInput").ap()
    vaug = nc.dram_tensor("vaug", [128, vf], _BF16, kind="ExternalInput").ap()
    out = nc.dram_tensor("out", [66, of], _F32, kind="ExternalOutput").ap()

    vc_f = vf // VCH               # 3168 cols per v chunk

    with tile.TileContext(nc) as tc, ExitStack() as ctx:
        qpool = ctx.enter_context(tc.tile_pool(name="qg", bufs=1))
        kpool = ctx.enter_context(tc.tile_pool(name="kg", bufs=1))
        vpool = ctx.enter_context(tc.tile_pool(name="vg", bufs=1))
        ppool = ctx.enter_context(tc.tile_pool(name="pt", bufs=5))
        opool = ctx.enter_context(tc.tile_pool(name="og", bufs=1))
        spoolA = ctx.enter_context(tc.tile_pool(name="spA", bufs=2, space="PSUM"))
        spoolB = ctx.enter_context(tc.tile_pool(name="spB", bufs=2, space="PSUM"))

        # warmup exp so the ACT table load attaches to a dep-free
        # instruction (the first real exp otherwise exceeds the
        # per-instruction sync-wait slot limit in walrus codegen)
        wpool = ctx.enter_context(tc.tile_pool(name="warm", bufs=1))
        win = wpool.tile([128, 8], _F32, tag="win")
        wout = wpool.tile([128, 8], _F32, tag="wout")
        nc.vector.memset(win[:], 0.0)
        nc.scalar.activation(wout[:], win[:], _Exp)

        import contextlib
        loop_cm = tc.For_i(0, loop_reps, 1) if loop_reps else contextlib.nullcontext()
        with loop_cm:
            qt, vt = [], [None] * VCH
            for i, (lo, hi) in enumerate(QCHUNKS):
                qg = qpool.tile([QROWS, (hi - lo + 1) * N], _BF16,
                                name=f"qg{i}")
                nc.sync.dma_start(qg[:], qaug[:, lo * N:(hi + 1) * N])
                qt.append(qg)
            # scalar (ACT HWDGE) ring order: k head first so compute can
            # start, v0 next (block 0's AV needs it), then the rest.
            kt = [None] * len(QCHUNKS)
            for o in (0, -1, 1, -2, 2, 3, 4):     # negative = v chunk
                if o < 0:
                    ch = -o - 1
                    vg = vpool.tile([128, vc_f], _BF16, name=f"vg{ch}")
                    nc.scalar.dma_start(vg[:],
                                        vaug[:, ch * vc_f:(ch + 1) * vc_f])
                    vt[ch] = vg
                else:
                    lo, hi = QCHUNKS[o]
                    kg = kpool.tile([QROWS, (hi - lo + 1) * N], _BF16,
                                    name=f"kg{o}")
                    nc.scalar.dma_start(kg[:], kaug[:, lo * N:(hi + 1) * N])
                    kt[o] = kg

            def qk_ap(p):
                for i, (lo, hi) in enumerate(QCHUNKS):
                    if lo <= p <= hi:
                        return i, (p - lo) * N
                raise AssertionError(p)

            def o_range(blk):
                for i, (lo, hi) in enumerate(OSTORES):
                    if lo <= blk <= hi:
                        return i, lo, hi
                raise AssertionError(blk)

            def v_ap(p, c):
                idx = (4 * p + c) * VA
                return vt[idx // vc_f][:, idx % vc_f:idx % vc_f + VA]

            def emit_av(st):
                blk, pts, av, ogt = st
                # A chain forward (pt[A] long done); B chain reversed so its
                # first matmul reads the DVE fast-exp chunk, not ACT's.
                for i in range(4):
                    for j in range(2):
                        c = i if j == 0 else 3 - i
                        base = 64 * j
                        nc.tensor.matmul(
                            av[base:base + VA, 0:N],
                            lhsT=v_ap(2 * blk + j, c),
                            rhs=pts[j][:, N * c:N * c + N],
                            start=(i == 0), stop=(i == 3),
                        )
                _, olo, ohi = o_range(blk)
                col = (blk - olo) * N
                nc.vector.tensor_copy(ogt[0:97, col:col + N], av[0:97, 0:N])
                if blk == ohi:
                    nc.gpsimd.dma_start(out[0:33, olo * N:(ohi + 1) * N],
                                        ogt[0:33, :])
                    nc.gpsimd.dma_start(out[33:66, olo * N:(ohi + 1) * N],
                                        ogt[64:97, :])

            pending = None
            ogt = None
            for blk in range(NBLK):
                oi, olo, ohi = o_range(blk)
                if blk == olo:
                    ogt = opool.tile([97, (ohi - olo + 1) * N], _F32,
                                     name=f"og{oi}")
                pts, spBs = [], []
                for j in range(2):
                    p = 2 * blk + j
                    ci, fq = qk_ap(p)
                    qg = qt[ci]
                    kg = kt[ci]
                    spA = spoolA.tile([128, 3 * N], _F32)
                    spB = spoolB.tile([128, N], _F32)
                    for c in range(4):
                        dst = spA[:, N * c:N * c + N] if c < 3 else spB[:, 0:N]
                        nc.tensor.matmul(
                            dst,
                            lhsT=kg[:, fq + 128 * c:fq + 128 * c + 128],
                            rhs=qg[:, fq:fq + N],
                            start=True, stop=True,
                        )
                    pt = ppool.tile([128, 4 * N], _BF16)
                    nc.scalar.activation(pt[:, 0:A_COLS], spA[:, 0:A_COLS], _Exp)
                    nc.vector.tensor_scalar(
                        pt[:, A_COLS:4 * N].bitcast(_I16),
                        spB[:, 0:N],
                        FE_SCALE, FE_BIAS, _mult, _add)
                    pts.append(pt)
                    spBs.append(spB)
                    # AV(blk-1) targets spB(2*blk-1) = the buf pair 2*blk+1
                    # will reuse, so it must be emitted between the two
                    # pairs' QK+exp.
                    if j == 0 and pending is not None:
                        emit_av(pending)
                        pending = None
                pending = (blk, pts, spBs[1], ogt)
            emit_av(pending)

    nc.compile()
    return nc


def kernel(q, k, v, h, w, d,
           pos_proj_w, pos_proj_b,
           ln1_g, ln1_b, w1, b1,
           ln2_g, ln2_b, w2, b2,
           ln3_g, ln3_b, w3, b3):
    h, w, d = int(h), int(w), int(d)
    n = h * w * d
    assert n == N, f"kernel hardcoded for N={N}, got {n}"
    scale = np.float32(q.shape[-1] ** -0.5)

    q = np.asarray(q, np.float32)
    k = np.asarray(k, np.float32)
    v = np.asarray(v, np.float32)
    args = [np.asarray(a, np.float32) for a in (
        pos_proj_w, pos_proj_b, ln1_g, ln1_b, w1, b1,
        ln2_g, ln2_b, w2, b2, ln3_g, ln3_b, w3, b3)]
    pos = _pos_table(h, w, d, *args)

    coords = np.stack(np.meshgrid(np.arange(h), np.arange(w), np.arange(d),
                                  indexing='ij')).reshape(3, -1)
    s = coords.sum(axis=0)
    s_max = (h - 1) + (w - 1) + (d - 1)
    naug = s_max + 1                           # 22
    assert naug == NAUG
    bidx = np.arange(naug)

    bf = ml_dtypes.bfloat16
    Qrows = pos[(s[:, None] - bidx[None, :]) + s_max, :]     # (N, naug, HEADS)
    E = (s[:, None] == bidx[None, :]).astype(np.float32)     # (N, naug)

    qs = q * scale
    q_hi = qs.astype(bf)
    q_lo = (qs - q_hi.astype(np.float32)).astype(bf)
    k_hi = k.astype(bf)
    k_lo = (k - k_hi.astype(np.float32)).astype(bf)

    D = HEAD_DIM
    qaug_all = np.zeros((B, HEADS, QROWS, N), dtype=bf)
    qaug_all[:, :, 0:D] = q_hi.transpose(0, 1, 3, 2)
    qaug_all[:, :, D:2 * D] = q_hi.transpose(0, 1, 3, 2)
    qaug_all[:, :, 2 * D:3 * D] = q_lo.transpose(0, 1, 3, 2)
    qaug_all[:, :, 3 * D:3 * D + naug] = Qrows.transpose(2, 1, 0).astype(bf)[None]
    kaug_all = np.zeros((B, HEADS, QROWS, N), dtype=bf)
    kaug_all[:, :, 0:D] = k_hi.transpose(0, 1, 3, 2)
    kaug_all[:, :, D:2 * D] = k_lo.transpose(0, 1, 3, 2)
    kaug_all[:, :, 2 * D:3 * D] = k_hi.transpose(0, 1, 3, 2)
    kaug_all[:, :, 3 * D:3 * D + naug] = E.T.astype(bf)[None, None]
    vaug_all = np.ones((B, HEADS, N, VA), dtype=bf)
    vaug_all[:, :, :, 0:D] = v.astype(bf)     # col D is the ones column

    def pack_qk(a):   # [PAIRS, QROWS, N] -> [QROWS, PAIRS*N], pair-major free
        return np.ascontiguousarray(a.transpose(1, 0, 2).reshape(QROWS, -1))

    def pack_v(a):    # [PAIRS, N, VA] -> [128, PAIRS*4*VA], chunk-major free
        return np.ascontiguousarray(
            a.reshape(PAIRS * 4, 128, VA).transpose(1, 0, 2).reshape(128, -1))

    in_maps = []
    for c in range(NCORES):
        sl = slice(c * BPC, (c + 1) * BPC)
        in_maps.append({
            "qaug": pack_qk(qaug_all[sl].reshape(PAIRS, QROWS, N)),
            "kaug": pack_qk(kaug_all[sl].reshape(PAIRS, QROWS, N)),
            "vaug": pack_v(vaug_all[sl].reshape(PAIRS, N, VA)),
        })

    nc = _build_device_program()
    res = run_bass_kernel_spmd(nc, in_maps, list(range(NCORES)))

    out = np.empty((B, HEADS, n, HEAD_DIM), np.float32)
    for c in range(NCORES):
        oc = np.asarray(res.results[c]["out"])           # [66, PAIRS/2*N]
        arr = oc.reshape(66, PAIRS // 2, n).transpose(1, 0, 2)  # [blk, 66, n]
        arr = arr.reshape(PAIRS // 2, 2, VA, n).reshape(PAIRS, VA, n)
        norm = arr[:, :HEAD_DIM] / arr[:, HEAD_DIM:HEAD_DIM + 1]
        out[c * BPC:(c + 1) * BPC] = (
            norm.transpose(0, 2, 1).reshape(BPC, HEADS, n, HEAD_DIM))
    return out


# revision 42
# speedup vs baseline: 1.0135x; 1.0135x over previous
"""Trainium2 Bass kernel: windowed attention with dynamic positional bias.

out = softmax(scale*q@k^T + bias) @ v per (batch, head); the tiny pos-bias
MLP table (43x6) is computed on host and folded into the QK matmul via
augmented contraction channels:

  qaug[n] . kaug[m] = scale*q[n].k[m] + pos[s(n)-s(m)+OFF]     (exact)

qaug = [qs_hi, qs_hi, qs_lo, pos rows] / kaug = [k_hi, k_lo, k_hi,
onehot(s(m))] in bf16, K=118.  Matmuls with K<~118 partition rows are NOT
faster and can be catastrophically slower on this stack: K=54
row-alternating tiles measured 2.6x slower, K=65 full-base 1.6x slower,
despite identical streaming width — keep operands near 128 partitions.
The hi/lo split is numerically free (rel_err 1.31e-2 vs 1.50e-2 plain).

Softmax exp is split between ScalarE (exact exp LUT over S' chunks 0-2
in spA) and VectorE (Schraudolph fast-exp over chunk 3 in spB: I16 =
l*184.665 + 16250.5 bit-reinterpreted as bf16 = e^l, ~2.5% sawtooth on
25% of the weights).  The split must stay at whole key-chunks: a
column-split of a chunk divides by QUERY, concentrating fast-exp to 50%
of some query rows' softmax terms (measured rel_err 2.31e-2, FAIL), and
sharing an spA bank between ACT and DVE readers also serializes the two
engines (Tile's bank tracker) — both hit when A_COLS=1280 was tried.

Layout (vs the 115us baseline):
- DMA is batched into ~1.5MB mega-transfers: each transfer pays ~2us of
  fixed completion latency on its FIFO ring, so the baseline's 12-group
  streaming (24+ transfers/ring) was latency-bound at ~70-85us/ring.
  Whole qaug/kaug/vaug stay resident in SBUF.  Rings: sync=qaug x4,
  scalar(ACT HWDGE)=kaug x4 + vaug x2, gpsimd=out stores x8.  (Moving
  loads to gpsimd/SWDGE or outs to sync measured far slower - keep this
  assignment.)
- AV runs as a 2-pair block: pair A accumulates at PSUM partitions 0-32,
  pair B at 64-96 of the SAME bank (pair B's consumed spB bank).
  start=True pending-zero clears are per-partition, so the interleaved
  chains don't clobber each other.  One DVE eviction copy [0:97] per
  block (f32 PSUM copies run 1x, so halving the copy count saves
  ~330ns/pair).
- Pair B's AV chain runs c3..c0 so its first matmul reads the DVE
  fast-exp output (ready early) instead of waiting on the ACT exp.

Data parallel: 8 cores x 8 batches = 48 (b,h) pairs/core.
"""

import sys

for _p in ("/opt/trn_rl_repo",):
    if _p not in sys.path:
        sys.path.insert(0, _p)

from contextlib import ExitStack

import ml_dtypes
import numpy as np

import concourse.bacc as bacc
import concourse.bass as bass
import concourse.tile as tile
from concourse import mybir
from concourse.bass_utils import run_bass_kernel_spmd

B, HEADS, HEAD_DIM = 64, 6, 32
NCORES = 8
BPC = B // NCORES              # batches per core
PAIRS = BPC * HEADS            # 48 (b,h) pairs per core
NBLK = PAIRS // 2              # 24 two-pair AV blocks
N = 512                        # sequence positions (h*w*d)
NAUG = 22                      # bias channels (s in 0..21)
QROWS = 128                    # qaug/kaug partition rows (hi/lo + bias = 118)
VA = HEAD_DIM + 1              # [v, ones] columns
A_COLS = 1536                  # exp cols on ACT (spA); 512 on DVE fast-exp (spB)
QCH = 4                        # q/k DMA chunks (12 pairs each)
PPC = PAIRS // QCH             # pairs per q/k chunk
VCH = 2                        # v DMA chunks (24 pairs each)
OCH = 4                        # out store quarters (6 blocks each)
BPQ = NBLK // OCH              # blocks per out quarter
FE_SCALE = 184.6649652337873   # 128 * log2(e)
FE_BIAS = 16250.5              # 127*128 + c; rint cast on DVE, c tuned on data

_BF16 = mybir.dt.bfloat16
_F32 = mybir.dt.float32
_I16 = mybir.dt.int16

_Exp = mybir.ActivationFunctionType.Exp
_mult = mybir.AluOpType.mult
_add = mybir.AluOpType.add


def _ln(x, g, b, eps=1e-5):
    mu = x.mean(axis=-1, keepdims=True)
    var = x.var(axis=-1, keepdims=True)
    return (x - mu) / np.sqrt(var + eps) * g + b


def _pos_table(h, w, d, pos_proj_w, pos_proj_b, ln1_g, ln1_b, w1, b1,
               ln2_g, ln2_b, w2, b2, ln3_g, ln3_b, w3, b3):
    bh = np.arange(1 - h, h, dtype=np.float32)
    bw = np.arange(1 - w, w, dtype=np.float32)
    bd = np.arange(1 - d, d, dtype=np.float32)
    mesh = np.stack(np.meshgrid(bh, bw, bd, indexing='ij')).reshape(3, -1).T
    x = mesh.astype(np.float32) @ pos_proj_w + pos_proj_b
    x = np.maximum(_ln(x, ln1_g, ln1_b), 0) @ w1 + b1
    x = np.maximum(_ln(x, ln2_g, ln2_b), 0) @ w2 + b2
    return (np.maximum(_ln(x, ln3_g, ln3_b), 0) @ w3 + b3).astype(np.float32)


def _build_device_program(loop_reps=None):
    """loop_reps: wrap the body in a device-side For_i (timing harness)."""
    nc = bacc.Bacc("TRN2", target_bir_lowering=False, debug=False)

    qf = PAIRS * N                 # qaug/kaug dram: [QROWS, qf] bf16
    vf = PAIRS * 4 * VA            # v dram: [128, vf] bf16
    of = NBLK * N                  # out dram: [66, of] f32 (2 pairs/block)

    qaug = nc.dram_tensor("qaug", [QROWS, qf], _BF16, kind="ExternalInput").ap()
    kaug = nc.dram_tensor("kaug", [QROWS, qf], _BF16, kind="ExternalInput").ap()
    vaug = nc.dram_tensor("vaug", [128, vf], _BF16, kind="ExternalInput").ap()
    out = nc.dram_tensor("out", [66, of], _F32, kind="ExternalOutput").ap()

    qc_f = PPC * N                 # 6144 cols per q/k chunk
    vc_f = vf // VCH               # 3168 cols per v chunk
    oq_f = BPQ * N                 # 3072 cols per out quarter

    with tile.TileContext(nc) as tc, ExitStack() as ctx:
        qpool = ctx.enter_context(tc.tile_pool(name="qg", bufs=1))
        kpool = ctx.enter_context(tc.tile_pool(name="kg", bufs=1))
        vpool = ctx.enter_context(tc.tile_pool(name="vg", bufs=1))
        ppool = ctx.enter_context(tc.tile_pool(name="pt", bufs=5))
        opool = ctx.enter_context(tc.tile_pool(name="og", bufs=OCH))
        spoolA = ctx.enter_context(tc.tile_pool(name="spA", bufs=2, space="PSUM"))
        spoolB = ctx.enter_context(tc.tile_pool(name="spB", bufs=2, space="PSUM"))

        # warmup exp so the ACT table load attaches to a dep-free
        # instruction (the first real exp otherwise exceeds the
        # per-instruction sync-wait slot limit in walrus codegen)
        wpool = ctx.enter_context(tc.tile_pool(name="warm", bufs=1))
        win = wpool.tile([128, 8], _F32, tag="win")
        wout = wpool.tile([128, 8], _F32, tag="wout")
        nc.vector.memset(win[:], 0.0)
        nc.scalar.activation(wout[:], win[:], _Exp)

        import contextlib
        loop_cm = tc.For_i(0, loop_reps, 1) if loop_reps else contextlib.nullcontext()
        with loop_cm:
            qt, kt, vt = [], [], []
            for ch in range(QCH):
                qg = qpool.tile([QROWS, qc_f], _BF16, name=f"qg{ch}")
                nc.sync.dma_start(qg[:], qaug[:, ch * qc_f:(ch + 1) * qc_f])
                qt.append(qg)
            # scalar (ACT HWDGE) ring order: k0 first so compute can start,
            # then both v chunks (block 0's AV needs v early), then k1-3.
            kg = kpool.tile([QROWS, qc_f], _BF16, name="kg0")
            nc.scalar.dma_start(kg[:], kaug[:, 0:qc_f])
            kt.append(kg)
            for ch in range(VCH):
                vg = vpool.tile([128, vc_f], _BF16, name=f"vg{ch}")
                nc.scalar.dma_start(vg[:], vaug[:, ch * vc_f:(ch + 1) * vc_f])
                vt.append(vg)
            for ch in range(1, QCH):
                kg = kpool.tile([QROWS, qc_f], _BF16, name=f"kg{ch}")
                nc.scalar.dma_start(kg[:], kaug[:, ch * qc_f:(ch + 1) * qc_f])
                kt.append(kg)

            def v_ap(p, c):
                idx = (4 * p + c) * VA
                return vt[idx // vc_f][:, idx % vc_f:idx % vc_f + VA]

            def emit_av(st):
                blk, pts, av, ogt = st
                # A chain forward (pt[A] long done); B chain reversed so its
                # first matmul reads the DVE fast-exp chunk, not ACT's.
                for i in range(4):
                    for j in range(2):
                        c = i if j == 0 else 3 - i
                        base = 64 * j
                        nc.tensor.matmul(
                            av[base:base + VA, 0:N],
                            lhsT=v_ap(2 * blk + j, c),
                            rhs=pts[j][:, N * c:N * c + N],
                            start=(i == 0), stop=(i == 3),
                        )
                col = (blk % BPQ) * N
                nc.vector.tensor_copy(ogt[0:97, col:col + N], av[0:97, 0:N])
                if blk % BPQ == BPQ - 1:
                    qi = blk // BPQ
                    nc.gpsimd.dma_start(out[0:33, qi * oq_f:(qi + 1) * oq_f],
                                        ogt[0:33, :])
                    nc.gpsimd.dma_start(out[33:66, qi * oq_f:(qi + 1) * oq_f],
                                        ogt[64:97, :])

            pending = None
            ogt = None
            for blk in range(NBLK):
                if blk % BPQ == 0:
                    ogt = opool.tile([97, oq_f], _F32)
                pts, spBs = [], []
                for j in range(2):
                    p = 2 * blk + j
                    qg = qt[p // PPC]
                    kg = kt[p // PPC]
                    fq = (p % PPC) * N
                    spA = spoolA.tile([128, 3 * N], _F32)
                    spB = spoolB.tile([128, N], _F32)
                    for c in range(4):
                        dst = spA[:, N * c:N * c + N] if c < 3 else spB[:, 0:N]
                        nc.tensor.matmul(
                            dst,
                            lhsT=kg[:, fq + 128 * c:fq + 128 * c + 128],
                            rhs=qg[:, fq:fq + N],
                            start=True, stop=True,
                        )
                    pt = ppool.tile([128, 4 * N], _BF16)
                    nc.scalar.activation(pt[:, 0:A_COLS], spA[:, 0:A_COLS], _Exp)
                    nc.vector.tensor_scalar(
                        pt[:, A_COLS:4 * N].bitcast(_I16),
                        spB[:, 0:N],
                        FE_SCALE, FE_BIAS, _mult, _add)
                    pts.append(pt)
                    spBs.append(spB)
                    # AV(blk-1) targets spB(2*blk-1) = the buf pair 2*blk+1
                    # will reuse, so it must be emitted between the two
                    # pairs' QK+exp.
                    if j == 0 and pending is not None:
                        emit_av(pending)
                        pending = None
                pending = (blk, pts, spBs[1], ogt)
            emit_av(pending)

    nc.compile()
    return nc


def kernel(q, k, v, h, w, d,
           pos_proj_w, pos_proj_b,
           ln1_g, ln1_b, w1, b1,
           ln2_g, ln2_b, w2, b2,
           ln3_g, ln3_b, w3, b3):
    h, w, d = int(h), int(w), int(d)
    n = h * w * d
    assert n == N, f"kernel hardcoded for N={N}, got {n}"
    scale = np.float32(q.shape[-1] ** -0.5)

    q = np.asarray(q, np.float32)
    k = np.asarray(k, np.float32)
    v = np.asarray(v, np.float32)
    args = [np.asarray(a, np.float32) for a in (
        pos_proj_w, pos_proj_b, ln1_g, ln1_b, w1, b1,
        ln2_g, ln2_b, w2, b2, ln3_g, ln3_b, w3, b3)]
    pos = _pos_table(h, w, d, *args)

    coords = np.stack(np.meshgrid(np.arange(h), np.arange(w), np.arange(d),
                                  indexing='ij')).reshape(3, -1)
    s = coords.sum(axis=0)
    s_max = (h - 1) + (w - 1) + (d - 1)
    naug = s_max + 1                           # 22
    assert naug == NAUG
    bidx = np.arange(naug)

    bf = ml_dtypes.bfloat16
    Qrows = pos[(s[:, None] - bidx[None, :]) + s_max, :]     # (N, naug, HEADS)
    E = (s[:, None] == bidx[None, :]).astype(np.float32)     # (N, naug)

    qs = q * scale
    q_hi = qs.astype(bf)
    q_lo = (qs - q_hi.astype(np.float32)).astype(bf)
    k_hi = k.astype(bf)
    k_lo = (k - k_hi.astype(np.float32)).astype(bf)

    D = HEAD_DIM
    qaug_all = np.zeros((B, HEADS, QROWS, N), dtype=bf)
    qaug_all[:, :, 0:D] = q_hi.transpose(0, 1, 3, 2)
    qaug_all[:, :, D:2 * D] = q_hi.transpose(0, 1, 3, 2)
    qaug_all[:, :, 2 * D:3 * D] = q_lo.transpose(0, 1, 3, 2)
    qaug_all[:, :, 3 * D:3 * D + naug] = Qrows.transpose(2, 1, 0).astype(bf)[None]
    kaug_all = np.zeros((B, HEADS, QROWS, N), dtype=bf)
    kaug_all[:, :, 0:D] = k_hi.transpose(0, 1, 3, 2)
    kaug_all[:, :, D:2 * D] = k_lo.transpose(0, 1, 3, 2)
    kaug_all[:, :, 2 * D:3 * D] = k_hi.transpose(0, 1, 3, 2)
    kaug_all[:, :, 3 * D:3 * D + naug] = E.T.astype(bf)[None, None]
    vaug_all = np.ones((B, HEADS, N, VA), dtype=bf)
    vaug_all[:, :, :, 0:D] = v.astype(bf)     # col D is the ones column

    def pack_qk(a):   # [PAIRS, QROWS, N] -> [QROWS, PAIRS*N], pair-major free
        return np.ascontiguousarray(a.transpose(1, 0, 2).reshape(QROWS, -1))

    def pack_v(a):    # [PAIRS, N, VA] -> [128, PAIRS*4*VA], chunk-major free
        return np.ascontiguousarray(
            a.reshape(PAIRS * 4, 128, VA).transpose(1, 0, 2).reshape(128, -1))

    in_maps = []
    for c in range(NCORES):
        sl = slice(c * BPC, (c + 1) * BPC)
        in_maps.append({
            "qaug": pack_qk(qaug_all[sl].reshape(PAIRS, QROWS, N)),
            "kaug": pack_qk(kaug_all[sl].reshape(PAIRS, QROWS, N)),
            "vaug": pack_v(vaug_all[sl].reshape(PAIRS, N, VA)),
        })

    nc = _build_device_program()
    res = run_bass_kernel_spmd(nc, in_maps, list(range(NCORES)))

    out = np.empty((B, HEADS, n, HEAD_DIM), np.float32)
    for c in range(NCORES):
        oc = np.asarray(res.results[c]["out"])           # [66, PAIRS/2*N]
        arr = oc.reshape(66, PAIRS // 2, n).transpose(1, 0, 2)  # [blk, 66, n]
        arr = arr.reshape(PAIRS // 2, 2, VA, n).reshape(PAIRS, VA, n)
        norm = arr[:, :HEAD_DIM] / arr[:, HEAD_DIM:HEAD_DIM + 1]
        out[c * BPC:(c + 1) * BPC] = (
            norm.transpose(0, 2, 1).reshape(BPC, HEADS, n, HEAD_DIM))
    return out
